# revision 14
# baseline (speedup 1.0000x reference)
"""Trainium2 Bass kernel for nn_AutoformerBase (sparse_attention).

Algorithm (algebraically reduced from the reference):
  mean_value[b, tau] = (1/D) sum_d corr(q_proj_d, k_proj_d)[tau]
                     = sum_{t,j} rho[b,t,j] * k_in[b,(t-tau)%L,j],
    where rho = q_in @ A and A = (Wq @ Wk^T)/D   (bq/bk only shift
    mean_value by a per-batch constant -> no effect on top-k or softmax).
  index = top6 of mean over b of mean_value  (one tiny AllReduce)
  tmp_corr = softmax(mean_value[:, index])
  out = sum_k tmp_corr[:,k] * Z[:, (t+tau_k)%L, :] + (bv@Wo + bo),
    where Z = v_in @ (Wv @ Wo)  (rolls commute with the right-side matmul,
    softmax weights sum to 1 for the bias term).

Device strategy (per core, data-parallel over batch, 4 batches/core):
  - bf16 everywhere on the PE (validated: top-6 indices and softmax
    weights are stable for this input distribution; final rel err ~1e-2
    vs 2e-2 gate). bf16 stationaries load 2x faster + FWL.
  - Loop nests ordered so each LDWEIGHTS covers >=1024 moving columns
    (weight load fully hidden in the PE pipeline).
  - mean_value via Gram diag-sums: G[t',t] = sum_j kT[j,t'] rho[j,t],
    evacuated PSUM->SBUF in bf16 through *sheared* DMA writes so that
    each column of the piece buffer holds one circulant diagonal;
    partition-reduced directly into a persistent [4, L] PSUM delta
    accumulator via ones-column stationaries (one column per batch).
  - All Z = vT @ Wc matmuls emitted after stats so the PE chews on Z
    while the AllReduce runs; aggregation tail split 3 batches on PE
    (weighted-identity matmuls over a doubled Z) + 1 batch on DVE.
"""
import math
from contextlib import ExitStack
import numpy as np
from ml_dtypes import bfloat16

import concourse.bass as bass
import concourse.mybir as mybir
import concourse.tile as tile
from concourse import bacc
from concourse.bass import ds
from concourse.tile import TileContext
from concourse.bass_utils import run_bass_kernel_spmd

B, L, D = 32, 1024, 512
NCORES = 8
BLOC = B // NCORES          # 4 batches per core
TOPK = 6
NPE = 3                     # batches aggregated on the PE (rest on DVE)
F32 = mybir.dt.float32
F32R = mybir.dt.float32r
BF16 = mybir.dt.bfloat16
U32 = mybir.dt.uint32
ALU = mybir.AluOpType
AFT = mybir.ActivationFunctionType

_CACHE = {}

NIC = D // 128           # 4 chunks of contraction
NJC = D // 128           # 4 chunks of output-feature rows
NTC = L // 512           # 2 free-dim chunks of 512
NTR = L // 128           # 8 row-blocks of t'


def _fold_segments(tr):
    """Split the 1152-wide sheared piece into (k0, t0, w, h) segments:
    pb column k0..k0+w maps to tau t0..t0+w in psum bank h."""
    base = (-128 - 128 * tr) % L
    segs = []
    k0 = 0
    while k0 < 1152:
        t0 = (base + k0) % L
        w = min(1152 - k0, L - t0)
        while w > 0:
            h = t0 // 512
            w2 = min(w, (h + 1) * 512 - t0)
            segs.append((k0, t0, w2, h))
            k0 += w2
            t0 += w2
            w -= w2
            if t0 == L:
                t0 = 0
    return segs


def _build():
    nc = bacc.Bacc("TRN2", target_bir_lowering=False)

    qT_d = nc.dram_tensor("qT", [BLOC, D, L], BF16, kind="ExternalInput")
    kT_d = nc.dram_tensor("kT", [BLOC, D, L], BF16, kind="ExternalInput")
    vT_d = nc.dram_tensor("vT", [BLOC, D, L], BF16, kind="ExternalInput")
    A_d = nc.dram_tensor("A", [D, D], BF16, kind="ExternalInput")
    Wc_d = nc.dram_tensor("Wc", [D, D], BF16, kind="ExternalInput")
    I_d = nc.dram_tensor("I128", [128, 128], BF16, kind="ExternalInput")
    selrow_d = nc.dram_tensor("selrow", [4, BLOC * 128], BF16,
                              kind="ExternalInput")
    outT_d = nc.dram_tensor("outT", [BLOC, D, L], F32, kind="ExternalOutput")
    cc_in = nc.dram_tensor("cc_in", [1, L], F32)
    cc_out = nc.dram_tensor("cc_out", [1, L], F32, addr_space="Shared")

    with TileContext(nc) as tc, ExitStack() as ctx:
        consts = ctx.enter_context(tc.tile_pool(name="consts", bufs=1))
        qk_pool = ctx.enter_context(tc.tile_pool(name="qk", bufs=2))
        rho_pool = ctx.enter_context(tc.tile_pool(name="rho", bufs=2))
        vz_pool = ctx.enter_context(tc.tile_pool(name="vz", bufs=1))
        gst_pool = ctx.enter_context(tc.tile_pool(name="gst", bufs=2))
        pb_pool = ctx.enter_context(tc.tile_pool(name="pb", bufs=1))
        small = ctx.enter_context(tc.tile_pool(name="small", bufs=1))
        out_pool = ctx.enter_context(tc.tile_pool(name="out", bufs=4))
        ps_a = ctx.enter_context(tc.tile_pool(name="ps_a", bufs=2, space="PSUM"))
        ps_d = ctx.enter_context(tc.tile_pool(name="ps_d", bufs=1, space="PSUM"))
        ps_g = ctx.enter_context(tc.tile_pool(name="ps_g", bufs=2, space="PSUM"))

        # ---- constants ----
        A_sb = consts.tile([128, NIC, D], BF16)
        Wc_sb = consts.tile([128, NIC, D], BF16)
        nc.sync.dma_start(out=A_sb, in_=A_d.rearrange("(ic p) j -> p ic j", p=128))
        nc.sync.dma_start(out=Wc_sb, in_=Wc_d.rearrange("(ic p) j -> p ic j", p=128))
        I_sb = consts.tile([128, 128], BF16)
        nc.sync.dma_start(out=I_sb, in_=I_d.ap())

        # fold stationaries: sel[:, b, :] is [128, 4] with column b = ones
        sel = consts.tile([128, BLOC, 4], BF16)
        nc.vector.memset(sel, 0.0)
        for b in range(BLOC):
            nc.vector.memset(sel[:, b, b:b + 1], 1.0)
        # broadcast stationaries: selrow[:, b, :] is [4, 128] with row b = ones
        selrow = consts.tile([4, BLOC, 128], BF16)
        nc.sync.dma_start(out=selrow, in_=selrow_d.rearrange(
            "p (b c) -> p b c", b=BLOC))

        # persistent sheared piece buffers: borders stay zero across reuse
        pbs = []
        for i in range(2):
            pb_t = pb_pool.tile([128, 1152], BF16, tag=f"pb{i}", name=f"pb{i}")
            nc.vector.memset(pb_t, 0.0)
            pbs.append(pb_t)

        # persistent [4, L] delta accumulator: 2 psum banks
        delta_ps = [ps_d.tile([4, 512], F32, tag=f"d{h}", name=f"delta{h}")
                    for h in range(2)]
        delta_started = [False, False]

        # ---- input DMAs (sync queue, in prefetch order) ----
        qTs, kTs, vTs = {}, {}, {}
        for b in range(BLOC):
            for ic in range(NIC):
                t = qk_pool.tile([128, L], BF16, tag=f"q{ic}", name=f"qT{b}_{ic}")
                nc.sync.dma_start(out=t, in_=qT_d.ap()[b, ic * 128:(ic + 1) * 128])
                qTs[(b, ic)] = t
            for ic in range(NIC):
                t = qk_pool.tile([128, L], BF16, tag=f"k{ic}", name=f"kT{b}_{ic}")
                nc.sync.dma_start(out=t, in_=kT_d.ap()[b, ic * 128:(ic + 1) * 128])
                kTs[(b, ic)] = t
            vt = vz_pool.tile([128, NIC, L], BF16, tag=f"v{b}", name=f"vT{b}")
            nc.sync.dma_start(out=vt, in_=vT_d.ap()[b].rearrange(
                "(ic p) t -> p ic t", p=128))
            vTs[b] = vt

        # ================= stats phase =================
        pb_i = 0
        deferred_fold = [None]  # (b, tr, pb) emitted one step later

        # per-bank index of the last fold segment (for stop flags)
        last_seg = {}
        for i, (k0, t0, w, h) in enumerate(_fold_segments(NTR - 1)):
            last_seg[h] = i

        def emit_fold():
            item = deferred_fold[0]
            if item is None:
                return
            b, tr, pb = item
            deferred_fold[0] = None
            for i, (k0, t0, w, h) in enumerate(_fold_segments(tr)):
                is_last = (b == BLOC - 1) and (tr == NTR - 1) and (last_seg[h] == i)
                nc.tensor.matmul(
                    delta_ps[h][:, t0 - h * 512:t0 - h * 512 + w],
                    lhsT=sel[:, b, :],
                    rhs=pb[:, k0:k0 + w],
                    start=(not delta_started[h]), stop=is_last,
                    skip_group_check=True)
                delta_started[h] = True

        for b in range(BLOC):
            # rho[j, t] = sum_i A[i, j] * qT[i, t]; stationary A(ic,jc)
            # covers 1024 moving columns per load.
            rho = rho_pool.tile([128, NJC, L], BF16, tag="rho", name=f"rho{b}")
            for jc in range(NJC):
                ps = ps_a.tile([128, L], F32, tag="pa", name=f"rps{b}_{jc}")
                for ic in range(NIC):
                    for tcc in range(NTC):
                        nc.tensor.matmul(
                            ps[:, tcc * 512:(tcc + 1) * 512],
                            lhsT=A_sb[:, ic, jc * 128:(jc + 1) * 128],
                            rhs=qTs[(b, ic)][:, tcc * 512:(tcc + 1) * 512],
                            start=(ic == 0), stop=(ic == NIC - 1),
                            skip_group_check=True)
                nc.scalar.copy(rho[:, jc, :], ps)

            # Gram row-blocks; stationary kT(jc, tr-block) covers 1024 cols.
            for tr in range(NTR):
                ps = ps_a.tile([128, L], F32, tag="pa", name=f"gps{b}_{tr}")
                for jc in range(NJC):
                    for tcc in range(NTC):
                        nc.tensor.matmul(
                            ps[:, tcc * 512:(tcc + 1) * 512],
                            lhsT=kTs[(b, jc)][:, tr * 128:(tr + 1) * 128],
                            rhs=rho[:, jc, tcc * 512:(tcc + 1) * 512],
                            start=(jc == 0), stop=(jc == NJC - 1),
                            skip_group_check=True)
                emit_fold()
                gst = gst_pool.tile([128, L], BF16, tag="gst")
                if tr % 2 == 0:
                    nc.scalar.copy(gst, ps)
                else:
                    nc.vector.tensor_copy(gst, ps)
                # sheared spill: pb[p, 128 - p + c] = gst[p, c]
                pb = pbs[pb_i % 2]
                pb_i += 1
                shear = bass.AP(tensor=pb.tensor, offset=pb.offset + 128,
                                ap=[[1152 - 1, 128], [1, L]])
                nc.gpsimd.dma_start(out=shear, in_=gst)
                deferred_fold[0] = (b, tr, pb)
        emit_fold()

        # ================= collective trigger =================
        mv4 = small.tile([4, L], F32)
        for h in range(2):
            nc.vector.tensor_copy(mv4[:, h * 512:(h + 1) * 512], delta_ps[h])
        bs_sb = small.tile([1, L], F32)
        nc.gpsimd.tensor_reduce(out=bs_sb, in_=mv4, axis=mybir.AxisListType.C,
                                op=ALU.add)
        nc.sync.dma_start(out=cc_in.ap(), in_=bs_sb)
        nc.gpsimd.collective_compute(
            "AllReduce", ALU.add,
            replica_groups=[list(range(NCORES))],
            ins=[cc_in.ap()], outs=[cc_out.ap()])
        bm = small.tile([1, L], F32)
        nc.sync.dma_start(out=bm, in_=cc_out.ap())

        # ================= Z phase (overlaps the AllReduce) ============
        # stationary Wc(ic,jc) covers 2 batches x 512 = 1024 cols per load
        z_tiles = {}
        for b in range(BLOC):
            z_tiles[b] = vz_pool.tile([128, NJC, 2 * L], BF16, tag=f"z{b}",
                                      name=f"Z{b}")
        for b0, b1 in ((0, 1), (2, 3)):
            for jc in range(NJC):
                for tcc in range(NTC):
                    ps = ps_a.tile([128, L], F32, tag="pa",
                                   name=f"zps{b0}_{jc}_{tcc}")
                    for ic in range(NIC):
                        for b in (b0, b1):
                            nc.tensor.matmul(
                                ps[:, (b - b0) * 512:(b - b0 + 1) * 512],
                                lhsT=Wc_sb[:, ic, jc * 128:(jc + 1) * 128],
                                rhs=vTs[b][:, ic, tcc * 512:(tcc + 1) * 512],
                                start=(ic == 0), stop=(ic == NIC - 1),
                                skip_group_check=True)
                    for b in (b0, b1):
                        dst = z_tiles[b][:, jc, tcc * 512:(tcc + 1) * 512]
                        src = ps[:, (b - b0) * 512:(b - b0 + 1) * 512]
                        if b == b0:
                            nc.scalar.copy(dst, src)
                        else:
                            nc.vector.tensor_copy(dst, src)
            for b in (b0, b1):
                nc.scalar.dma_start(out=z_tiles[b][:, :, L:2 * L],
                                    in_=z_tiles[b][:, :, 0:L])

        # ================= top-k + weights =================
        vals8 = small.tile([1, 8], F32)
        idx8 = small.tile([1, 8], U32)
        nc.vector.max_with_indices(vals8, idx8, bm)

        # tau registers for engines that need dynamic offsets
        tau_v = []
        for k in range(TOPK):
            r = nc.vector.alloc_register(f"tau{k}")
            nc.vector.reg_load(r, idx8[0:1, k:k + 1])
            base_sv = nc.snap(r, min_val=0, max_val=L - 1)
            r2 = nc.vector.alloc_register(f"tau{k}_hi")
            nc.vector.reg_load(r2, idx8[0:1, k:k + 1])
            nc.vector.reg_add(r2, r2, 512)
            hi_sv = nc.snap(r2, min_val=512, max_val=L - 1 + 512)
            tau_v.append((base_sv, hi_sv))
        tau_t = []
        for k in range(TOPK):
            r = nc.tensor.alloc_register(f"tau_t{k}")
            nc.tensor.reg_load(r, idx8[0:1, k:k + 1])
            base_sv = nc.snap(r, min_val=0, max_val=L - 1)
            r2 = nc.tensor.alloc_register(f"tau_t{k}_hi")
            nc.tensor.reg_load(r2, idx8[0:1, k:k + 1])
            nc.tensor.reg_add(r2, r2, 512)
            hi_sv = nc.snap(r2, min_val=512, max_val=L - 1 + 512)
            tau_t.append((base_sv, hi_sv))

        # gather mv4[:, tau_k] and softmax over k, vectorized on partitions 0-3
        w4 = small.tile([4, 8], F32)
        for k in range(TOPK):
            nc.vector.tensor_copy(w4[:, k:k + 1], mv4[:, ds(tau_v[k][0], 1)])
        w6 = w4[:, 0:TOPK]
        mx = small.tile([4, 1], F32)
        nc.vector.tensor_reduce(out=mx, in_=w6, axis=mybir.AxisListType.X,
                                op=ALU.max)
        negmx = small.tile([4, 1], F32)
        nc.vector.tensor_scalar(out=negmx, in0=mx, scalar1=-1.0,
                                scalar2=None, op0=ALU.mult)
        ex = small.tile([4, 8], F32)
        sm = small.tile([4, 1], F32)
        nc.scalar.activation(ex[:, 0:TOPK], w6, AFT.Exp, bias=negmx,
                             accum_out=sm)
        rc = small.tile([4, 1], F32)
        nc.vector.reciprocal(rc, sm)
        wnb = small.tile([4, 8], BF16)
        nc.vector.memset(wnb, 0.0)
        nc.vector.tensor_scalar(out=wnb[:, 0:TOPK], in0=ex[:, 0:TOPK],
                                scalar1=rc, scalar2=None, op0=ALU.mult)

        # broadcast w to all 128 partitions: w_bc[p, b*8+k] = w[b, k]
        psw = ps_d.tile([128, 32], F32, tag="d0", name="psw")
        for b in range(BLOC):
            nc.tensor.matmul(psw[:, b * 8:(b + 1) * 8], lhsT=selrow[:, b, :],
                             rhs=wnb, start=True, stop=True,
                             skip_group_check=True)
        w_bc = small.tile([128, BLOC, 8], F32)
        nc.scalar.copy(w_bc, psw)

        # weighted identities for the PE aggregation path
        wIs = {}
        for b in range(NPE):
            wI = []
            for k in range(TOPK):
                t = small.tile([128, 128], BF16, tag=f"wI{b}_{k}",
                               name=f"wI{b}_{k}")
                nc.vector.tensor_scalar(out=t, in0=I_sb,
                                        scalar1=w_bc[:, b, k:k + 1],
                                        scalar2=None, op0=ALU.mult)
                wI.append(t)
            wIs[b] = wI

        # ================= aggregation =================
        # DVE batches first (vector engine works while PE aggregates)
        for b in range(NPE, BLOC):
            Z = z_tiles[b]
            for jc in range(NJC):
                for tcc in range(NTC):
                    acc = out_pool.tile([128, 512], BF16, tag="dacc",
                                        name=f"dacc{b}_{jc}_{tcc}")
                    nc.vector.tensor_scalar(
                        out=acc,
                        in0=Z[:, jc, ds(tau_v[0][tcc], 512)],
                        scalar1=w_bc[:, b, 0:1], scalar2=None, op0=ALU.mult)
                    for k in range(1, TOPK - 1):
                        nc.vector.scalar_tensor_tensor(
                            out=acc,
                            in0=Z[:, jc, ds(tau_v[k][tcc], 512)],
                            scalar=w_bc[:, b, k:k + 1],
                            in1=acc, op0=ALU.mult, op1=ALU.add)
                    accf = out_pool.tile([128, 512], F32, tag="daccf",
                                         name=f"daccf{b}_{jc}_{tcc}")
                    nc.vector.scalar_tensor_tensor(
                        out=accf,
                        in0=Z[:, jc, ds(tau_v[TOPK - 1][tcc], 512)],
                        scalar=w_bc[:, b, TOPK - 1:TOPK],
                        in1=acc, op0=ALU.mult, op1=ALU.add)
                    nc.sync.dma_start(
                        out=outT_d.ap()[b, jc * 128:(jc + 1) * 128,
                                        tcc * 512:(tcc + 1) * 512],
                        in_=accf)

        # PE batches: 6 weighted-identity matmuls per psum tile
        for b in range(NPE):
            Z = z_tiles[b]
            for jc in range(NJC):
                for tcc in range(NTC):
                    ps = ps_g.tile([128, 512], F32, tag="agg",
                                   name=f"aps{b}_{jc}_{tcc}")
                    for k in range(TOPK):
                        nc.tensor.matmul(
                            ps,
                            lhsT=wIs[b][k],
                            rhs=Z[:, jc, ds(tau_t[k][tcc], 512)],
                            start=(k == 0), stop=(k == TOPK - 1),
                            skip_group_check=True)
                    acc = out_pool.tile([128, 512], F32, tag="pacc",
                                        name=f"pacc{b}_{jc}_{tcc}")
                    nc.scalar.copy(acc, ps)
                    nc.sync.dma_start(
                        out=outT_d.ap()[b, jc * 128:(jc + 1) * 128,
                                        tcc * 512:(tcc + 1) * 512],
                        in_=acc)

    nc.compile()
    return nc


def _get_nc():
    if "nc" not in _CACHE:
        _CACHE["nc"] = _build()
    return _CACHE["nc"]


def _run(inputs, trace=False, tmpdir=None):
    q_in = np.ascontiguousarray(inputs["q_in"], dtype=np.float32)
    k_in = np.ascontiguousarray(inputs["k_in"], dtype=np.float32)
    v_in = np.ascontiguousarray(inputs["v_in"], dtype=np.float32)
    Wq, Wk, Wv, Wo = inputs["Wq"], inputs["Wk"], inputs["Wv"], inputs["Wo"]
    bv, bo = inputs["bv"], inputs["bo"]

    A = ((Wq.astype(np.float64) @ Wk.astype(np.float64).T) / D).astype(bfloat16)
    Wc = (Wv.astype(np.float64) @ Wo.astype(np.float64)).astype(bfloat16)
    c_row = (bv.astype(np.float64) @ Wo.astype(np.float64) + bo).astype(np.float32)

    qT = np.ascontiguousarray(q_in.transpose(0, 2, 1).astype(bfloat16))
    kT = np.ascontiguousarray(k_in.transpose(0, 2, 1).astype(bfloat16))
    vT = np.ascontiguousarray(v_in.transpose(0, 2, 1).astype(bfloat16))
    I128 = np.eye(128, dtype=bfloat16)
    selrow = np.zeros((4, BLOC, 128), dtype=bfloat16)
    for b in range(BLOC):
        selrow[b, b, :] = 1
    selrow = selrow.reshape(4, BLOC * 128)

    nc = _get_nc()
    in_maps = []
    for c in range(NCORES):
        sl = slice(c * BLOC, (c + 1) * BLOC)
        in_maps.append({
            "qT": qT[sl], "kT": kT[sl], "vT": vT[sl],
            "A": A, "Wc": Wc, "I128": I128, "selrow": selrow,
        })
    res = run_bass_kernel_spmd(nc, in_maps, list(range(NCORES)),
                               trace=trace, tmpdir=tmpdir)
    outT = np.concatenate([r["outT"] for r in res.results], axis=0)  # (B, D, L)
    out = outT.transpose(0, 2, 1) + c_row[None, None, :]
    return np.ascontiguousarray(out, dtype=np.float32), res


def kernel(q_in, k_in, v_in, Wq, bq, Wk, bk, Wv, bv, Wo, bo):
    out, _ = _run(dict(q_in=q_in, k_in=k_in, v_in=v_in, Wq=Wq, bq=bq,
                       Wk=Wk, bk=bk, Wv=Wv, bv=bv, Wo=Wo, bo=bo))
    return out


# revision 17
# speedup vs baseline: 1.3771x; 1.3771x over previous
"""Trainium2 Bass kernel for nn_AutoformerBase (sparse_attention).

Algorithm (algebraically reduced from the reference):
  mean_value[b, tau] = (1/D) sum_d corr(q_proj_d, k_proj_d)[tau]
                     = sum_{t,j} rho[b,t,j] * k_in[b,(t-tau)%L,j],
    where rho = q_in @ A and A = (Wq @ Wk^T)/D   (bq/bk only shift
    mean_value by a per-batch constant -> no effect on top-k or softmax).
  index = top6 of mean over b of mean_value  (one tiny AllReduce)
  tmp_corr = softmax(mean_value[:, index])
  out = sum_k tmp_corr[:,k] * Z[:, (t+tau_k)%L, :] + (bv@Wo + bo),
    where Z = v_in @ (Wv @ Wo)  (rolls commute with the right-side matmul,
    softmax weights sum to 1 for the bias term).

Device strategy (per core, data-parallel over batch, 4 batches/core):
  - bf16 everywhere on the PE (validated: top-6 indices and softmax
    weights are stable for this input distribution; final rel err ~1e-2
    vs 2e-2 gate). bf16 stationaries load 2x faster + FWL.
  - Loop nests ordered so each LDWEIGHTS covers >=1024 moving columns
    (weight load fully hidden in the PE pipeline).
  - mean_value via Gram diag-sums: G[t',t] = sum_j kT[j,t'] rho[j,t],
    evacuated PSUM->SBUF in bf16 through *sheared* DMA writes so that
    each column of the piece buffer holds one circulant diagonal;
    partition-reduced directly into a persistent [4, L] PSUM delta
    accumulator via ones-column stationaries (one column per batch).
  - All Z = vT @ Wc matmuls emitted after stats so the PE chews on Z
    while the AllReduce runs; aggregation tail split 3 batches on PE
    (weighted-identity matmuls over a doubled Z) + 1 batch on DVE.
"""
import math
from contextlib import ExitStack
import numpy as np
from ml_dtypes import bfloat16

import concourse.bass as bass
import concourse.mybir as mybir
import concourse.tile as tile
from concourse import bacc
from concourse.bass import ds
from concourse.tile import TileContext
from concourse.bass_utils import run_bass_kernel_spmd

B, L, D = 32, 1024, 512
NCORES = 8
BLOC = B // NCORES          # 4 batches per core
TOPK = 6
NPE = 3                     # batches aggregated on the PE (rest on DVE)
F32 = mybir.dt.float32
F32R = mybir.dt.float32r
BF16 = mybir.dt.bfloat16
U32 = mybir.dt.uint32
ALU = mybir.AluOpType
AFT = mybir.ActivationFunctionType

_CACHE = {}

NIC = D // 128           # 4 chunks of contraction
NJC = D // 128           # 4 chunks of output-feature rows
NTC = L // 512           # 2 free-dim chunks of 512
NTR = L // 128           # 8 row-blocks of t'


def _fold_segments(tr):
    """Split the 1152-wide sheared piece into (k0, t0, w, h) segments:
    pb column k0..k0+w maps to tau t0..t0+w in psum bank h."""
    base = (-128 - 128 * tr) % L
    segs = []
    k0 = 0
    while k0 < 1152:
        t0 = (base + k0) % L
        w = min(1152 - k0, L - t0)
        while w > 0:
            h = t0 // 512
            w2 = min(w, (h + 1) * 512 - t0)
            segs.append((k0, t0, w2, h))
            k0 += w2
            t0 += w2
            w -= w2
            if t0 == L:
                t0 = 0
    return segs


def _build():
    nc = bacc.Bacc("TRN2", target_bir_lowering=False)

    qT_d = nc.dram_tensor("qT", [BLOC, D, L], BF16, kind="ExternalInput")
    kT_d = nc.dram_tensor("kT", [BLOC, D, L], BF16, kind="ExternalInput")
    vT_d = nc.dram_tensor("vT", [BLOC, D, L], BF16, kind="ExternalInput")
    A_d = nc.dram_tensor("A", [D, D], BF16, kind="ExternalInput")
    Wc_d = nc.dram_tensor("Wc", [D, D], BF16, kind="ExternalInput")
    I_d = nc.dram_tensor("I128", [128, 128], BF16, kind="ExternalInput")
    selrow_d = nc.dram_tensor("selrow", [4, BLOC * 128], BF16,
                              kind="ExternalInput")
    outT_d = nc.dram_tensor("outT", [BLOC, D, L], F32, kind="ExternalOutput")
    cc_in = nc.dram_tensor("cc_in", [1, L], F32)
    cc_out = nc.dram_tensor("cc_out", [1, L], F32, addr_space="Shared")

    with TileContext(nc) as tc, ExitStack() as ctx:
        consts = ctx.enter_context(tc.tile_pool(name="consts", bufs=1))
        qk_pool = ctx.enter_context(tc.tile_pool(name="qk", bufs=2))
        rho_pool = ctx.enter_context(tc.tile_pool(name="rho", bufs=2))
        vz_pool = ctx.enter_context(tc.tile_pool(name="vz", bufs=1))
        gst_pool = ctx.enter_context(tc.tile_pool(name="gst", bufs=2))
        pb_pool = ctx.enter_context(tc.tile_pool(name="pb", bufs=1))
        small = ctx.enter_context(tc.tile_pool(name="small", bufs=1))
        out_pool = ctx.enter_context(tc.tile_pool(name="out", bufs=4))
        ps_a = ctx.enter_context(tc.tile_pool(name="ps_a", bufs=2, space="PSUM"))
        ps_d = ctx.enter_context(tc.tile_pool(name="ps_d", bufs=1, space="PSUM"))
        ps_g = ctx.enter_context(tc.tile_pool(name="ps_g", bufs=2, space="PSUM"))

        # ---- constants ----
        A_sb = consts.tile([128, NIC, D], BF16)
        Wc_sb = consts.tile([128, NIC, D], BF16)
        nc.sync.dma_start(out=A_sb, in_=A_d.rearrange("(ic p) j -> p ic j", p=128))
        nc.sync.dma_start(out=Wc_sb, in_=Wc_d.rearrange("(ic p) j -> p ic j", p=128))
        I_sb = consts.tile([128, 128], BF16)
        nc.sync.dma_start(out=I_sb, in_=I_d.ap())

        # fold stationaries: sel[:, b, :] is [128, 36] with column b = ones
        # (accumulates batch b's diag-sums on psum partition b) and column
        # 32 = ones (accumulates the batch-TOTAL on psum partition 32 --
        # partition 32 because engine APs must start 32-aligned).
        sel = consts.tile([128, BLOC, 36], BF16)
        nc.vector.memset(sel, 0.0)
        for b in range(BLOC):
            nc.vector.memset(sel[:, b, b:b + 1], 1.0)
            nc.vector.memset(sel[:, b, 32:33], 1.0)
        # broadcast stationaries: selrow[:, b, :] is [4, 128] with row b = ones
        selrow = consts.tile([4, BLOC, 128], BF16)
        nc.sync.dma_start(out=selrow, in_=selrow_d.rearrange(
            "p (b c) -> p b c", b=BLOC))

        # persistent sheared piece buffers: borders stay zero across reuse
        pbs = []
        for i in range(2):
            pb_t = pb_pool.tile([128, 1152], BF16, tag=f"pb{i}", name=f"pb{i}")
            nc.vector.memset(pb_t, 0.0)
            pbs.append(pb_t)

        # persistent [36, L] delta accumulator: 2 psum banks
        # (rows 0-3: per-batch diag-sums; row 32: batch total)
        delta_ps = [ps_d.tile([36, 512], F32, tag=f"d{h}", name=f"delta{h}")
                    for h in range(2)]
        delta_started = [False, False]

        # ---- input DMAs (sync queue, in prefetch order) ----
        qTs, kTs, vTs = {}, {}, {}
        for b in range(BLOC):
            for ic in range(NIC):
                t = qk_pool.tile([128, L], BF16, tag=f"q{ic}", name=f"qT{b}_{ic}")
                nc.sync.dma_start(out=t, in_=qT_d.ap()[b, ic * 128:(ic + 1) * 128])
                qTs[(b, ic)] = t
            for ic in range(NIC):
                t = qk_pool.tile([128, L], BF16, tag=f"k{ic}", name=f"kT{b}_{ic}")
                nc.sync.dma_start(out=t, in_=kT_d.ap()[b, ic * 128:(ic + 1) * 128])
                kTs[(b, ic)] = t
            vt = vz_pool.tile([128, NIC, L], BF16, tag=f"v{b}", name=f"vT{b}")
            nc.sync.dma_start(out=vt, in_=vT_d.ap()[b].rearrange(
                "(ic p) t -> p ic t", p=128))
            vTs[b] = vt

        # ================= stats phase =================
        pb_i = 0
        deferred_fold = [None]  # (b, tr, pb) emitted one step later

        # per-bank index of the last fold segment (for stop flags)
        last_seg = {}
        for i, (k0, t0, w, h) in enumerate(_fold_segments(NTR - 1)):
            last_seg[h] = i

        def emit_fold():
            item = deferred_fold[0]
            if item is None:
                return
            b, tr, pb = item
            deferred_fold[0] = None
            for i, (k0, t0, w, h) in enumerate(_fold_segments(tr)):
                is_last = (b == BLOC - 1) and (tr == NTR - 1) and (last_seg[h] == i)
                nc.tensor.matmul(
                    delta_ps[h][:, t0 - h * 512:t0 - h * 512 + w],
                    lhsT=sel[:, b, :],
                    rhs=pb[:, k0:k0 + w],
                    start=(not delta_started[h]), stop=is_last,
                    skip_group_check=True)
                delta_started[h] = True

        for b in range(BLOC):
            # rho[j, t] = sum_i A[i, j] * qT[i, t]; stationary A(ic,jc)
            # covers 1024 moving columns per load.
            rho = rho_pool.tile([128, NJC, L], BF16, tag="rho", name=f"rho{b}")
            for jc in range(NJC):
                ps = ps_a.tile([128, L], F32, tag="pa", name=f"rps{b}_{jc}")
                for ic in range(NIC):
                    for tcc in range(NTC):
                        nc.tensor.matmul(
                            ps[:, tcc * 512:(tcc + 1) * 512],
                            lhsT=A_sb[:, ic, jc * 128:(jc + 1) * 128],
                            rhs=qTs[(b, ic)][:, tcc * 512:(tcc + 1) * 512],
                            start=(ic == 0), stop=(ic == NIC - 1),
                            skip_group_check=True)
                nc.scalar.copy(rho[:, jc, :], ps)

            # Gram row-blocks; stationary kT(jc, tr-block) covers 1024 cols.
            for tr in range(NTR):
                ps = ps_a.tile([128, L], F32, tag="pa", name=f"gps{b}_{tr}")
                for jc in range(NJC):
                    for tcc in range(NTC):
                        nc.tensor.matmul(
                            ps[:, tcc * 512:(tcc + 1) * 512],
                            lhsT=kTs[(b, jc)][:, tr * 128:(tr + 1) * 128],
                            rhs=rho[:, jc, tcc * 512:(tcc + 1) * 512],
                            start=(jc == 0), stop=(jc == NJC - 1),
                            skip_group_check=True)
                emit_fold()
                gst = gst_pool.tile([128, L], BF16, tag="gst")
                if tr % 2 == 0:
                    nc.scalar.copy(gst, ps)
                else:
                    nc.vector.tensor_copy(gst, ps)
                # sheared spill: pb[p, 128 - p + c] = gst[p, c]
                pb = pbs[pb_i % 2]
                pb_i += 1
                shear = bass.AP(tensor=pb.tensor, offset=pb.offset + 128,
                                ap=[[1152 - 1, 128], [1, L]])
                nc.gpsimd.dma_start(out=shear, in_=gst)
                deferred_fold[0] = (b, tr, pb)
        emit_fold()

        # ================= collective trigger =================
        bs_sb = small.tile([1, L], F32)
        for h in range(2):
            nc.scalar.copy(bs_sb[:, h * 512:(h + 1) * 512],
                           delta_ps[h][32:33, :])
        nc.sync.dma_start(out=cc_in.ap(), in_=bs_sb)
        mv4 = small.tile([4, L], F32)
        for h in range(2):
            nc.vector.tensor_copy(mv4[:, h * 512:(h + 1) * 512],
                                  delta_ps[h][0:4, :])
        nc.gpsimd.collective_compute(
            "AllReduce", ALU.add,
            replica_groups=[list(range(NCORES))],
            ins=[cc_in.ap()], outs=[cc_out.ap()])
        bm = small.tile([1, L], F32)
        nc.sync.dma_start(out=bm, in_=cc_out.ap())

        # ================= Z phase (overlaps the AllReduce) ============
        # stationary Wc(ic,jc) covers 2 batches x 512 = 1024 cols per load
        z_tiles = {}
        for b in range(BLOC):
            z_tiles[b] = vz_pool.tile([128, NJC, 2 * L], BF16, tag=f"z{b}",
                                      name=f"Z{b}")
        for b0, b1 in ((0, 1), (2, 3)):
            for jc in range(NJC):
                for tcc in range(NTC):
                    ps = ps_a.tile([128, L], F32, tag="pa",
                                   name=f"zps{b0}_{jc}_{tcc}")
                    for ic in range(NIC):
                        for b in (b0, b1):
                            nc.tensor.matmul(
                                ps[:, (b - b0) * 512:(b - b0 + 1) * 512],
                                lhsT=Wc_sb[:, ic, jc * 128:(jc + 1) * 128],
                                rhs=vTs[b][:, ic, tcc * 512:(tcc + 1) * 512],
                                start=(ic == 0), stop=(ic == NIC - 1),
                                skip_group_check=True)
                    for b in (b0, b1):
                        dst = z_tiles[b][:, jc, tcc * 512:(tcc + 1) * 512]
                        src = ps[:, (b - b0) * 512:(b - b0 + 1) * 512]
                        if b == b0:
                            nc.scalar.copy(dst, src)
                        else:
                            nc.vector.tensor_copy(dst, src)
            for b in (b0, b1):
                nc.scalar.dma_start(out=z_tiles[b][:, :, L:2 * L],
                                    in_=z_tiles[b][:, :, 0:L])

        # ================= top-k + weights =================
        vals8 = small.tile([1, 8], F32)
        idx8 = small.tile([1, 8], U32)
        nc.vector.max_with_indices(vals8, idx8, bm)

        # tau registers for engines that need dynamic offsets
        tau_v = []
        for k in range(TOPK):
            r = nc.vector.alloc_register(f"tau{k}")
            nc.vector.reg_load(r, idx8[0:1, k:k + 1])
            base_sv = nc.snap(r, min_val=0, max_val=L - 1)
            r2 = nc.vector.alloc_register(f"tau{k}_hi")
            nc.vector.reg_load(r2, idx8[0:1, k:k + 1])
            nc.vector.reg_add(r2, r2, 512)
            hi_sv = nc.snap(r2, min_val=512, max_val=L - 1 + 512)
            tau_v.append((base_sv, hi_sv))
        tau_t = []
        for k in range(TOPK):
            r = nc.tensor.alloc_register(f"tau_t{k}")
            nc.tensor.reg_load(r, idx8[0:1, k:k + 1])
            base_sv = nc.snap(r, min_val=0, max_val=L - 1)
            r2 = nc.tensor.alloc_register(f"tau_t{k}_hi")
            nc.tensor.reg_load(r2, idx8[0:1, k:k + 1])
            nc.tensor.reg_add(r2, r2, 512)
            hi_sv = nc.snap(r2, min_val=512, max_val=L - 1 + 512)
            tau_t.append((base_sv, hi_sv))

        # gather mv4[:, tau_k] and softmax over k, vectorized on partitions 0-3
        w4 = small.tile([4, 8], F32)
        for k in range(TOPK):
            nc.vector.tensor_copy(w4[:, k:k + 1], mv4[:, ds(tau_v[k][0], 1)])
        w6 = w4[:, 0:TOPK]
        mx = small.tile([4, 1], F32)
        nc.vector.tensor_reduce(out=mx, in_=w6, axis=mybir.AxisListType.X,
                                op=ALU.max)
        negmx = small.tile([4, 1], F32)
        nc.vector.tensor_scalar(out=negmx, in0=mx, scalar1=-1.0,
                                scalar2=None, op0=ALU.mult)
        ex = small.tile([4, 8], F32)
        sm = small.tile([4, 1], F32)
        nc.scalar.activation(ex[:, 0:TOPK], w6, AFT.Exp, bias=negmx,
                             accum_out=sm)
        rc = small.tile([4, 1], F32)
        nc.vector.reciprocal(rc, sm)
        wnb = small.tile([4, 8], BF16)
        nc.vector.memset(wnb, 0.0)
        nc.vector.tensor_scalar(out=wnb[:, 0:TOPK], in0=ex[:, 0:TOPK],
                                scalar1=rc, scalar2=None, op0=ALU.mult)

        # broadcast w to all 128 partitions: w_bc[p, b*8+k] = w[b, k]
        psw = ps_d.tile([128, 32], F32, tag="d0", name="psw")
        for b in range(BLOC):
            nc.tensor.matmul(psw[:, b * 8:(b + 1) * 8], lhsT=selrow[:, b, :],
                             rhs=wnb, start=True, stop=True,
                             skip_group_check=True)
        w_bc = small.tile([128, BLOC, 8], F32)
        nc.scalar.copy(w_bc, psw)

        # weighted identities for the PE aggregation path
        wIs = {}
        for b in range(NPE):
            wI = []
            for k in range(TOPK):
                t = small.tile([128, 128], BF16, tag=f"wI{b}_{k}",
                               name=f"wI{b}_{k}")
                nc.vector.tensor_scalar(out=t, in0=I_sb,
                                        scalar1=w_bc[:, b, k:k + 1],
                                        scalar2=None, op0=ALU.mult)
                wI.append(t)
            wIs[b] = wI

        # ================= aggregation =================
        # DVE batches first (vector engine works while PE aggregates)
        for b in range(NPE, BLOC):
            Z = z_tiles[b]
            for jc in range(NJC):
                for tcc in range(NTC):
                    acc = out_pool.tile([128, 512], BF16, tag="dacc",
                                        name=f"dacc{b}_{jc}_{tcc}")
                    nc.vector.tensor_scalar(
                        out=acc,
                        in0=Z[:, jc, ds(tau_v[0][tcc], 512)],
                        scalar1=w_bc[:, b, 0:1], scalar2=None, op0=ALU.mult)
                    for k in range(1, TOPK - 1):
                        nc.vector.scalar_tensor_tensor(
                            out=acc,
                            in0=Z[:, jc, ds(tau_v[k][tcc], 512)],
                            scalar=w_bc[:, b, k:k + 1],
                            in1=acc, op0=ALU.mult, op1=ALU.add)
                    accf = out_pool.tile([128, 512], F32, tag="daccf",
                                         name=f"daccf{b}_{jc}_{tcc}")
                    nc.vector.scalar_tensor_tensor(
                        out=accf,
                        in0=Z[:, jc, ds(tau_v[TOPK - 1][tcc], 512)],
                        scalar=w_bc[:, b, TOPK - 1:TOPK],
                        in1=acc, op0=ALU.mult, op1=ALU.add)
                    nc.sync.dma_start(
                        out=outT_d.ap()[b, jc * 128:(jc + 1) * 128,
                                        tcc * 512:(tcc + 1) * 512],
                        in_=accf)

        # PE batches: 6 weighted-identity matmuls per psum tile
        for b in range(NPE):
            Z = z_tiles[b]
            for jc in range(NJC):
                for tcc in range(NTC):
                    ps = ps_g.tile([128, 512], F32, tag="agg",
                                   name=f"aps{b}_{jc}_{tcc}")
                    for k in range(TOPK):
                        nc.tensor.matmul(
                            ps,
                            lhsT=wIs[b][k],
                            rhs=Z[:, jc, ds(tau_t[k][tcc], 512)],
                            start=(k == 0), stop=(k == TOPK - 1),
                            skip_group_check=True)
                    acc = out_pool.tile([128, 512], F32, tag="pacc",
                                        name=f"pacc{b}_{jc}_{tcc}")
                    nc.scalar.copy(acc, ps)
                    nc.sync.dma_start(
                        out=outT_d.ap()[b, jc * 128:(jc + 1) * 128,
                                        tcc * 512:(tcc + 1) * 512],
                        in_=acc)

    nc.compile()
    return nc


def _get_nc():
    if "nc" not in _CACHE:
        _CACHE["nc"] = _build()
    return _CACHE["nc"]


def _run(inputs, trace=False, tmpdir=None):
    q_in = np.ascontiguousarray(inputs["q_in"], dtype=np.float32)
    k_in = np.ascontiguousarray(inputs["k_in"], dtype=np.float32)
    v_in = np.ascontiguousarray(inputs["v_in"], dtype=np.float32)
    Wq, Wk, Wv, Wo = inputs["Wq"], inputs["Wk"], inputs["Wv"], inputs["Wo"]
    bv, bo = inputs["bv"], inputs["bo"]

    A = ((Wq.astype(np.float64) @ Wk.astype(np.float64).T) / D).astype(bfloat16)
    Wc = (Wv.astype(np.float64) @ Wo.astype(np.float64)).astype(bfloat16)
    c_row = (bv.astype(np.float64) @ Wo.astype(np.float64) + bo).astype(np.float32)

    qT = np.ascontiguousarray(q_in.transpose(0, 2, 1).astype(bfloat16))
    kT = np.ascontiguousarray(k_in.transpose(0, 2, 1).astype(bfloat16))
    vT = np.ascontiguousarray(v_in.transpose(0, 2, 1).astype(bfloat16))
    I128 = np.eye(128, dtype=bfloat16)
    selrow = np.zeros((4, BLOC, 128), dtype=bfloat16)
    for b in range(BLOC):
        selrow[b, b, :] = 1
    selrow = selrow.reshape(4, BLOC * 128)

    nc = _get_nc()
    in_maps = []
    for c in range(NCORES):
        sl = slice(c * BLOC, (c + 1) * BLOC)
        in_maps.append({
            "qT": qT[sl], "kT": kT[sl], "vT": vT[sl],
            "A": A, "Wc": Wc, "I128": I128, "selrow": selrow,
        })
    res = run_bass_kernel_spmd(nc, in_maps, list(range(NCORES)),
                               trace=trace, tmpdir=tmpdir)
    outT = np.concatenate([r["outT"] for r in res.results], axis=0)  # (B, D, L)
    out = outT.transpose(0, 2, 1) + c_row[None, None, :]
    return np.ascontiguousarray(out, dtype=np.float32), res


def kernel(q_in, k_in, v_in, Wq, bq, Wk, bk, Wv, bv, Wo, bo):
    out, _ = _run(dict(q_in=q_in, k_in=k_in, v_in=v_in, Wq=Wq, bq=bq,
                       Wk=Wk, bk=bk, Wv=Wv, bv=bv, Wo=Wo, bo=bo))
    return out


# revision 21
# speedup vs baseline: 1.4376x; 1.0439x over previous
"""Trainium2 Bass kernel for nn_AutoformerBase (sparse_attention).

Algorithm (algebraically reduced from the reference):
  mean_value[b, tau] = (1/D) sum_d corr(q_proj_d, k_proj_d)[tau]
                     = sum_{t,j} rho[b,t,j] * k_in[b,(t-tau)%L,j],
    where rho = q_in @ A and A = (Wq @ Wk^T)/D   (bq/bk only shift
    mean_value by a per-batch constant -> no effect on top-k or softmax).
  index = top6 of mean over b of mean_value  (one tiny AllReduce)
  tmp_corr = softmax(mean_value[:, index])
  out = sum_k tmp_corr[:,k] * Z[:, (t+tau_k)%L, :] + (bv@Wo + bo),
    where Z = v_in @ (Wv @ Wo)  (rolls commute with the right-side matmul,
    softmax weights sum to 1 for the bias term).

Device strategy (per core, data-parallel over batch, 4 batches/core):
  - bf16 everywhere on the PE (validated: top-6 indices and softmax
    weights are stable for this input distribution; final rel err ~1e-2
    vs 2e-2 gate). bf16 stationaries load 2x faster + FWL.
  - Loop nests ordered so each LDWEIGHTS covers >=1024 moving columns
    (weight load fully hidden in the PE pipeline).
  - mean_value via Gram diag-sums: G[t',t] = sum_j kT[j,t'] rho[j,t],
    evacuated PSUM->SBUF in bf16 through *sheared* DMA writes so that
    each column of the piece buffer holds one circulant diagonal;
    partition-reduced directly into a persistent [4, L] PSUM delta
    accumulator via ones-column stationaries (one column per batch).
  - All Z = vT @ Wc matmuls emitted after stats so the PE chews on Z
    while the AllReduce runs; aggregation tail split 3 batches on PE
    (weighted-identity matmuls over a doubled Z) + 1 batch on DVE.
"""
import math
from contextlib import ExitStack
import numpy as np
from ml_dtypes import bfloat16

import concourse.bass as bass
import concourse.mybir as mybir
import concourse.tile as tile
from concourse import bacc
from concourse.bass import ds
from concourse.tile import TileContext
from concourse.bass_utils import run_bass_kernel_spmd

B, L, D = 32, 1024, 512
NCORES = 8
BLOC = B // NCORES          # 4 batches per core
TOPK = 6
NPE = 3                     # batches aggregated on the PE (rest on DVE)
F32 = mybir.dt.float32
F32R = mybir.dt.float32r
BF16 = mybir.dt.bfloat16
U32 = mybir.dt.uint32
ALU = mybir.AluOpType
AFT = mybir.ActivationFunctionType

_CACHE = {}

NIC = D // 128           # 4 chunks of contraction
NJC = D // 128           # 4 chunks of output-feature rows
NTC = L // 512           # 2 free-dim chunks of 512
NTR = L // 128           # 8 row-blocks of t'


def _fold_segments(tr):
    """Split the 1152-wide sheared piece into (k0, t0, w, h) segments:
    pb column k0..k0+w maps to tau t0..t0+w in psum bank h."""
    base = (-128 - 128 * tr) % L
    segs = []
    k0 = 0
    while k0 < 1152:
        t0 = (base + k0) % L
        w = min(1152 - k0, L - t0)
        while w > 0:
            h = t0 // 512
            w2 = min(w, (h + 1) * 512 - t0)
            segs.append((k0, t0, w2, h))
            k0 += w2
            t0 += w2
            w -= w2
            if t0 == L:
                t0 = 0
    return segs


def _build():
    nc = bacc.Bacc("TRN2", target_bir_lowering=False)

    qT_d = nc.dram_tensor("qT", [BLOC, D, L], BF16, kind="ExternalInput")
    kT_d = nc.dram_tensor("kT", [BLOC, D, L], BF16, kind="ExternalInput")
    vT_d = nc.dram_tensor("vT", [BLOC, D, L], BF16, kind="ExternalInput")
    A_d = nc.dram_tensor("A", [D, D], BF16, kind="ExternalInput")
    Wc_d = nc.dram_tensor("Wc", [D, D], BF16, kind="ExternalInput")
    I_d = nc.dram_tensor("I128", [128, 128], BF16, kind="ExternalInput")
    selrow_d = nc.dram_tensor("selrow", [4, BLOC * 128], BF16,
                              kind="ExternalInput")
    outT_d = nc.dram_tensor("outT", [BLOC, D, L], F32, kind="ExternalOutput")
    cc_in = nc.dram_tensor("cc_in", [1, L], F32)
    cc_out = nc.dram_tensor("cc_out", [1, L], F32, addr_space="Shared")

    with TileContext(nc) as tc, ExitStack() as ctx:
        consts = ctx.enter_context(tc.tile_pool(name="consts", bufs=1))
        qk_pool = ctx.enter_context(tc.tile_pool(name="qk", bufs=3))
        rho_pool = ctx.enter_context(tc.tile_pool(name="rho", bufs=2))
        vz_pool = ctx.enter_context(tc.tile_pool(name="vz", bufs=1))
        gst_pool = ctx.enter_context(tc.tile_pool(name="gst", bufs=2))
        pb_pool = ctx.enter_context(tc.tile_pool(name="pb", bufs=1))
        small = ctx.enter_context(tc.tile_pool(name="small", bufs=1))
        out_pool = ctx.enter_context(tc.tile_pool(name="out", bufs=2))
        ps_a = ctx.enter_context(tc.tile_pool(name="ps_a", bufs=2, space="PSUM"))
        ps_d = ctx.enter_context(tc.tile_pool(name="ps_d", bufs=1, space="PSUM"))
        ps_g = ctx.enter_context(tc.tile_pool(name="ps_g", bufs=2, space="PSUM"))

        # ---- constants (only A is needed before the stats phase) ----
        A_sb = consts.tile([128, NIC, D], BF16)
        Wc_sb = consts.tile([128, NIC, D], BF16)
        nc.sync.dma_start(out=A_sb, in_=A_d.rearrange("(ic p) j -> p ic j", p=128))
        I_sb = consts.tile([128, 128], BF16)

        # fold stationaries: sel[:, b, :] is [128, 36] with column b = ones
        # (accumulates batch b's diag-sums on psum partition b) and column
        # 32 = ones (accumulates the batch-TOTAL on psum partition 32 --
        # partition 32 because engine APs must start 32-aligned).
        sel = consts.tile([128, BLOC, 36], BF16)
        nc.vector.memset(sel, 0.0)
        for b in range(BLOC):
            nc.vector.memset(sel[:, b, b:b + 1], 1.0)
            nc.vector.memset(sel[:, b, 32:33], 1.0)
        # broadcast stationaries: selrow[:, b, :] is [4, 128] with row b = ones
        selrow = consts.tile([4, BLOC, 128], BF16)
        nc.sync.dma_start(out=selrow, in_=selrow_d.rearrange(
            "p (b c) -> p b c", b=BLOC))

        # persistent sheared piece buffers: borders stay zero across reuse
        pbs = []
        for i in range(2):
            pb_t = pb_pool.tile([128, 1152], BF16, tag=f"pb{i}", name=f"pb{i}")
            nc.vector.memset(pb_t, 0.0)
            pbs.append(pb_t)

        # persistent [36, L] delta accumulator: 2 psum banks
        # (rows 0-3: per-batch diag-sums; row 32: batch total)
        delta_ps = [ps_d.tile([36, 512], F32, tag=f"d{h}", name=f"delta{h}")
                    for h in range(2)]
        delta_started = [False, False]

        # ---- input DMAs (sync queue = in-order: q/k prefetch first, then
        # everything the Z phase needs, so vT never head-of-line-blocks the
        # stats-phase prefetch) ----
        qTs, kTs, vTs = {}, {}, {}
        for b in range(BLOC):
            for ic in range(NIC):
                t = qk_pool.tile([128, L], BF16, tag=f"q{ic}", name=f"qT{b}_{ic}")
                nc.sync.dma_start(out=t, in_=qT_d.ap()[b, ic * 128:(ic + 1) * 128])
                qTs[(b, ic)] = t
            for ic in range(NIC):
                t = qk_pool.tile([128, L], BF16, tag=f"k{ic}", name=f"kT{b}_{ic}")
                nc.sync.dma_start(out=t, in_=kT_d.ap()[b, ic * 128:(ic + 1) * 128])
                kTs[(b, ic)] = t
        nc.sync.dma_start(out=Wc_sb, in_=Wc_d.rearrange("(ic p) j -> p ic j", p=128))
        nc.sync.dma_start(out=I_sb, in_=I_d.ap())
        for b in range(BLOC):
            vt = vz_pool.tile([128, NIC, L], BF16, tag=f"v{b}", name=f"vT{b}")
            nc.sync.dma_start(out=vt, in_=vT_d.ap()[b].rearrange(
                "(ic p) t -> p ic t", p=128))
            vTs[b] = vt

        # ================= stats phase =================
        pb_i = 0
        deferred_fold = [None]  # (b, tr, pb) emitted one step later

        # per-bank index of the last fold segment (for stop flags)
        last_seg = {}
        for i, (k0, t0, w, h) in enumerate(_fold_segments(NTR - 1)):
            last_seg[h] = i

        def emit_fold():
            item = deferred_fold[0]
            if item is None:
                return
            b, tr, pb = item
            deferred_fold[0] = None
            for i, (k0, t0, w, h) in enumerate(_fold_segments(tr)):
                is_last = (b == BLOC - 1) and (tr == NTR - 1) and (last_seg[h] == i)
                nc.tensor.matmul(
                    delta_ps[h][:, t0 - h * 512:t0 - h * 512 + w],
                    lhsT=sel[:, b, :],
                    rhs=pb[:, k0:k0 + w],
                    start=(not delta_started[h]), stop=is_last,
                    skip_group_check=True)
                delta_started[h] = True

        for b in range(BLOC):
            # rho[j, t] = sum_i A[i, j] * qT[i, t]; stationary A(ic,jc)
            # covers 1024 moving columns per load.
            rho = rho_pool.tile([128, NJC, L], BF16, tag="rho", name=f"rho{b}")
            for jc in range(NJC):
                ps = ps_a.tile([128, L], F32, tag="pa", name=f"rps{b}_{jc}")
                for ic in range(NIC):
                    for tcc in range(NTC):
                        nc.tensor.matmul(
                            ps[:, tcc * 512:(tcc + 1) * 512],
                            lhsT=A_sb[:, ic, jc * 128:(jc + 1) * 128],
                            rhs=qTs[(b, ic)][:, tcc * 512:(tcc + 1) * 512],
                            start=(ic == 0), stop=(ic == NIC - 1),
                            skip_group_check=True)
                nc.scalar.copy(rho[:, jc, :], ps)

            # Gram row-blocks; stationary kT(jc, tr-block) covers 1024 cols.
            for tr in range(NTR):
                ps = ps_a.tile([128, L], F32, tag="pa", name=f"gps{b}_{tr}")
                for jc in range(NJC):
                    for tcc in range(NTC):
                        nc.tensor.matmul(
                            ps[:, tcc * 512:(tcc + 1) * 512],
                            lhsT=kTs[(b, jc)][:, tr * 128:(tr + 1) * 128],
                            rhs=rho[:, jc, tcc * 512:(tcc + 1) * 512],
                            start=(jc == 0), stop=(jc == NJC - 1),
                            skip_group_check=True)
                emit_fold()
                gst = gst_pool.tile([128, L], BF16, tag="gst")
                if tr % 2 == 0:
                    nc.scalar.copy(gst, ps)
                else:
                    nc.vector.tensor_copy(gst, ps)
                # sheared spill: pb[p, 128 - p + c] = gst[p, c]
                pb = pbs[pb_i % 2]
                pb_i += 1
                shear = bass.AP(tensor=pb.tensor, offset=pb.offset + 128,
                                ap=[[1152 - 1, 128], [1, L]])
                nc.gpsimd.dma_start(out=shear, in_=gst)
                deferred_fold[0] = (b, tr, pb)
        emit_fold()

        # ================= collective trigger =================
        bs_sb = small.tile([1, L], F32)
        for h in range(2):
            nc.scalar.copy(bs_sb[:, h * 512:(h + 1) * 512],
                           delta_ps[h][32:33, :])
        nc.sync.dma_start(out=cc_in.ap(), in_=bs_sb)
        mv4 = small.tile([4, L], F32)
        for h in range(2):
            nc.vector.tensor_copy(mv4[:, h * 512:(h + 1) * 512],
                                  delta_ps[h][0:4, :])
        nc.gpsimd.collective_compute(
            "AllReduce", ALU.add,
            replica_groups=[list(range(NCORES))],
            ins=[cc_in.ap()], outs=[cc_out.ap()])
        bm = small.tile([1, L], F32)
        nc.sync.dma_start(out=bm, in_=cc_out.ap())

        # ================= Z phase (overlaps the AllReduce) ============
        # stationary Wc(ic,jc) covers 2 batches x 512 = 1024 cols per load
        z_tiles = {}
        for b in range(BLOC):
            z_tiles[b] = vz_pool.tile([128, NJC, 2 * L], BF16, tag=f"z{b}",
                                      name=f"Z{b}")
        for b0, b1 in ((0, 1), (2, 3)):
            for jc in range(NJC):
                for tcc in range(NTC):
                    ps = ps_a.tile([128, L], F32, tag="pa",
                                   name=f"zps{b0}_{jc}_{tcc}")
                    for ic in range(NIC):
                        for b in (b0, b1):
                            nc.tensor.matmul(
                                ps[:, (b - b0) * 512:(b - b0 + 1) * 512],
                                lhsT=Wc_sb[:, ic, jc * 128:(jc + 1) * 128],
                                rhs=vTs[b][:, ic, tcc * 512:(tcc + 1) * 512],
                                start=(ic == 0), stop=(ic == NIC - 1),
                                skip_group_check=True)
                    for b in (b0, b1):
                        dst = z_tiles[b][:, jc, tcc * 512:(tcc + 1) * 512]
                        src = ps[:, (b - b0) * 512:(b - b0 + 1) * 512]
                        if b == b0:
                            nc.scalar.copy(dst, src)
                        else:
                            nc.vector.tensor_copy(dst, src)
            for b in (b0, b1):
                nc.scalar.dma_start(out=z_tiles[b][:, :, L:2 * L],
                                    in_=z_tiles[b][:, :, 0:L])

        # ================= top-k + weights =================
        vals8 = small.tile([1, 8], F32)
        idx8 = small.tile([1, 8], U32)
        nc.vector.max_with_indices(vals8, idx8, bm)

        # tau registers for engines that need dynamic offsets
        tau_v = []
        for k in range(TOPK):
            r = nc.vector.alloc_register(f"tau{k}")
            nc.vector.reg_load(r, idx8[0:1, k:k + 1])
            base_sv = nc.snap(r, min_val=0, max_val=L - 1)
            r2 = nc.vector.alloc_register(f"tau{k}_hi")
            nc.vector.reg_load(r2, idx8[0:1, k:k + 1])
            nc.vector.reg_add(r2, r2, 512)
            hi_sv = nc.snap(r2, min_val=512, max_val=L - 1 + 512)
            tau_v.append((base_sv, hi_sv))
        tau_t = []
        for k in range(TOPK):
            r = nc.tensor.alloc_register(f"tau_t{k}")
            nc.tensor.reg_load(r, idx8[0:1, k:k + 1])
            base_sv = nc.snap(r, min_val=0, max_val=L - 1)
            r2 = nc.tensor.alloc_register(f"tau_t{k}_hi")
            nc.tensor.reg_load(r2, idx8[0:1, k:k + 1])
            nc.tensor.reg_add(r2, r2, 512)
            hi_sv = nc.snap(r2, min_val=512, max_val=L - 1 + 512)
            tau_t.append((base_sv, hi_sv))

        # gather mv4[:, tau_k] and softmax over k, vectorized on partitions 0-3
        w4 = small.tile([4, 8], F32)
        for k in range(TOPK):
            nc.vector.tensor_copy(w4[:, k:k + 1], mv4[:, ds(tau_v[k][0], 1)])
        w6 = w4[:, 0:TOPK]
        mx = small.tile([4, 1], F32)
        nc.vector.tensor_reduce(out=mx, in_=w6, axis=mybir.AxisListType.X,
                                op=ALU.max)
        negmx = small.tile([4, 1], F32)
        nc.vector.tensor_scalar(out=negmx, in0=mx, scalar1=-1.0,
                                scalar2=None, op0=ALU.mult)
        ex = small.tile([4, 8], F32)
        sm = small.tile([4, 1], F32)
        nc.scalar.activation(ex[:, 0:TOPK], w6, AFT.Exp, bias=negmx,
                             accum_out=sm)
        rc = small.tile([4, 1], F32)
        nc.vector.reciprocal(rc, sm)
        wnb = small.tile([4, 8], BF16)
        nc.vector.memset(wnb, 0.0)
        nc.vector.tensor_scalar(out=wnb[:, 0:TOPK], in0=ex[:, 0:TOPK],
                                scalar1=rc, scalar2=None, op0=ALU.mult)

        # broadcast w to all 128 partitions: w_bc[p, b*8+k] = w[b, k]
        psw = ps_d.tile([128, 32], F32, tag="d0", name="psw")
        for b in range(BLOC):
            nc.tensor.matmul(psw[:, b * 8:(b + 1) * 8], lhsT=selrow[:, b, :],
                             rhs=wnb, start=True, stop=True,
                             skip_group_check=True)
        w_bc = small.tile([128, BLOC, 8], F32)
        nc.scalar.copy(w_bc, psw)

        # weighted identities for the PE aggregation path
        wIs = {}
        for b in range(NPE):
            wI = []
            for k in range(TOPK):
                t = small.tile([128, 128], BF16, tag=f"wI{b}_{k}",
                               name=f"wI{b}_{k}")
                nc.vector.tensor_scalar(out=t, in0=I_sb,
                                        scalar1=w_bc[:, b, k:k + 1],
                                        scalar2=None, op0=ALU.mult)
                wI.append(t)
            wIs[b] = wI

        # ================= aggregation =================
        # DVE batches first (vector engine works while PE aggregates)
        for b in range(NPE, BLOC):
            Z = z_tiles[b]
            for jc in range(NJC):
                for tcc in range(NTC):
                    acc = out_pool.tile([128, 512], BF16, tag="dacc",
                                        name=f"dacc{b}_{jc}_{tcc}")
                    nc.vector.tensor_scalar(
                        out=acc,
                        in0=Z[:, jc, ds(tau_v[0][tcc], 512)],
                        scalar1=w_bc[:, b, 0:1], scalar2=None, op0=ALU.mult)
                    for k in range(1, TOPK - 1):
                        nc.vector.scalar_tensor_tensor(
                            out=acc,
                            in0=Z[:, jc, ds(tau_v[k][tcc], 512)],
                            scalar=w_bc[:, b, k:k + 1],
                            in1=acc, op0=ALU.mult, op1=ALU.add)
                    accf = out_pool.tile([128, 512], F32, tag="daccf",
                                         name=f"daccf{b}_{jc}_{tcc}")
                    nc.vector.scalar_tensor_tensor(
                        out=accf,
                        in0=Z[:, jc, ds(tau_v[TOPK - 1][tcc], 512)],
                        scalar=w_bc[:, b, TOPK - 1:TOPK],
                        in1=acc, op0=ALU.mult, op1=ALU.add)
                    nc.sync.dma_start(
                        out=outT_d.ap()[b, jc * 128:(jc + 1) * 128,
                                        tcc * 512:(tcc + 1) * 512],
                        in_=accf)

        # PE batches: 6 weighted-identity matmuls per psum tile
        for b in range(NPE):
            Z = z_tiles[b]
            for jc in range(NJC):
                for tcc in range(NTC):
                    ps = ps_g.tile([128, 512], F32, tag="agg",
                                   name=f"aps{b}_{jc}_{tcc}")
                    for k in range(TOPK):
                        nc.tensor.matmul(
                            ps,
                            lhsT=wIs[b][k],
                            rhs=Z[:, jc, ds(tau_t[k][tcc], 512)],
                            start=(k == 0), stop=(k == TOPK - 1),
                            skip_group_check=True)
                    acc = out_pool.tile([128, 512], F32, tag="pacc",
                                        name=f"pacc{b}_{jc}_{tcc}")
                    nc.scalar.copy(acc, ps)
                    nc.sync.dma_start(
                        out=outT_d.ap()[b, jc * 128:(jc + 1) * 128,
                                        tcc * 512:(tcc + 1) * 512],
                        in_=acc)

    nc.compile()
    return nc


def _get_nc():
    if "nc" not in _CACHE:
        _CACHE["nc"] = _build()
    return _CACHE["nc"]


def _run(inputs, trace=False, tmpdir=None):
    q_in = np.ascontiguousarray(inputs["q_in"], dtype=np.float32)
    k_in = np.ascontiguousarray(inputs["k_in"], dtype=np.float32)
    v_in = np.ascontiguousarray(inputs["v_in"], dtype=np.float32)
    Wq, Wk, Wv, Wo = inputs["Wq"], inputs["Wk"], inputs["Wv"], inputs["Wo"]
    bv, bo = inputs["bv"], inputs["bo"]

    A = ((Wq.astype(np.float64) @ Wk.astype(np.float64).T) / D).astype(bfloat16)
    Wc = (Wv.astype(np.float64) @ Wo.astype(np.float64)).astype(bfloat16)
    c_row = (bv.astype(np.float64) @ Wo.astype(np.float64) + bo).astype(np.float32)

    qT = np.ascontiguousarray(q_in.transpose(0, 2, 1).astype(bfloat16))
    kT = np.ascontiguousarray(k_in.transpose(0, 2, 1).astype(bfloat16))
    vT = np.ascontiguousarray(v_in.transpose(0, 2, 1).astype(bfloat16))
    I128 = np.eye(128, dtype=bfloat16)
    selrow = np.zeros((4, BLOC, 128), dtype=bfloat16)
    for b in range(BLOC):
        selrow[b, b, :] = 1
    selrow = selrow.reshape(4, BLOC * 128)

    nc = _get_nc()
    in_maps = []
    for c in range(NCORES):
        sl = slice(c * BLOC, (c + 1) * BLOC)
        in_maps.append({
            "qT": qT[sl], "kT": kT[sl], "vT": vT[sl],
            "A": A, "Wc": Wc, "I128": I128, "selrow": selrow,
        })
    res = run_bass_kernel_spmd(nc, in_maps, list(range(NCORES)),
                               trace=trace, tmpdir=tmpdir)
    outT = np.concatenate([r["outT"] for r in res.results], axis=0)  # (B, D, L)
    out = outT.transpose(0, 2, 1) + c_row[None, None, :]
    return np.ascontiguousarray(out, dtype=np.float32), res


def kernel(q_in, k_in, v_in, Wq, bq, Wk, bk, Wv, bv, Wo, bo):
    out, _ = _run(dict(q_in=q_in, k_in=k_in, v_in=v_in, Wq=Wq, bq=bq,
                       Wk=Wk, bk=bk, Wv=Wv, bv=bv, Wo=Wo, bo=bo))
    return out
